# revision 3
# baseline (speedup 1.0000x reference)
"""AFNO (Adaptive Fourier Neural Operator) Trainium2 kernel, v3.

Data-parallel over batch: 32 batches -> 8 cores x 4 batches.
Per core: 4 batches x 2 cq-pair groups (c-quarter pairs = block pairs).
Pipeline per group (see make_consts for the packed stationaries):
  x --(gpsimd cast load, w on partitions)--> S2 Wfft (block-diag, 112x116)
    --> s1 DRAM bounce --(strided transposed reads)--> yh [(r,h) | w',c]
    --> S4 Hfft (paired psum banks) --> XF --> S5 PE corner turn (dual
    identity) --> XM [2c+r | q,w',h] --> M1 relu(.+b1) --> M2 softshrink
    (e1/e2 pair trick) --> S8 PE corner turn back --> S9 iH (half-zero
    stationaries) --> s4y DRAM bounce laid out (Hh,r,w',h,c) so the
    transpose cost sits on the stores and the reads back are a single
    nearly-contiguous DMA per h-chunk --> S11 iW --> out stores.

v3 structure (vs the v2 baseline, ~2.2x faster on HW, sim 0.87ms/core):
  - software-pipelined issue order across groups: x loads for group i+2
    and rot1 reads for i+1 are issued inside group i's middle stage so
    the in-order engine queues always have ready work; per-stage state
    in st[i].
  - PSUM->SBUF evictions paired into [128, 2, 512] tiles (one unified
    8-bank "A" ring, bufs=4) halving eviction instruction count; M2's
    softshrink pieces computed on 896-wide pairs.
  - constants packed host-side into 3 tiles / 3 DMAs.
  - DMAs greedily balanced across SP+Pool queues (est. ns per queue);
    evictions balanced between DVE and ACT.
"""
import numpy as np
import ml_dtypes
from contextlib import ExitStack

import concourse.bass as bass
import concourse.mybir as mybir
import concourse.tile as tile
from concourse import bacc
from concourse.bass_utils import run_bass_kernel_spmd
from concourse.masks import make_identity

H = 56
W = 56
WF = 29
C = 768
NB = 4
BS = 192
LAM = 0.01
NCORES = 8
B_FULL = 32
BPC = B_FULL // NCORES  # 4
NCH = 4                 # h-chunks per group
HCH = H // NCH          # 14
NHC = H * BS
NWC = WF * BS           # 5568
SPA = H * WF            # 1624 spatial per half

F32 = mybir.dt.float32
BF16 = mybir.dt.bfloat16
AF = mybir.ActivationFunctionType
ALU = mybir.AluOpType

BF = ml_dtypes.bfloat16

# DFT-pack column offsets
DFT_COLS = {}
_off = 0
for _name, _w in (("fw2", 116), ("fhr", 64), ("fhi", 64), ("ghra", 112),
                  ("ghrb", 112), ("ghia", 112), ("ghib", 112), ("gw2", 112),
                  ("idd2", 112)):
    DFT_COLS[_name] = (_off, _w)
    _off += _w
DFT_W = _off  # 916


def make_consts(w1, b1, w2, b2):
    """Pack DFT matrices and mixing weights/biases host-side (numpy)."""
    w = np.arange(W)
    wp = np.arange(WF)
    ang = 2 * np.pi * np.outer(wp, w) / W
    Cw = np.cos(ang) / np.sqrt(W)
    Sw = np.sin(ang) / np.sqrt(W)
    h = np.arange(H)
    angh = 2 * np.pi * np.outer(h, h) / H
    Ch = np.cos(angh) / np.sqrt(H)
    Sh = np.sin(angh) / np.sqrt(H)
    Chi, Shi = Ch, Sh
    alpha = np.full(WF, 2.0)
    alpha[0] = 1.0
    alpha[WF - 1] = 1.0
    A = (alpha[None, :] * np.cos(2 * np.pi * np.outer(w, wp) / W)) / np.sqrt(W)
    Bm = (-alpha[None, :] * np.sin(2 * np.pi * np.outer(w, wp) / W)) / np.sqrt(W)
    Bm[:, 0] = 0.0
    Bm[:, WF - 1] = 0.0

    # W-fft stationary, block-diag for the two stacked c-quarters.
    fwA = np.zeros((W, 58), np.float32)
    fwA[:, :WF] = Cw.T
    fwA[:, WF:] = -Sw.T
    fw2 = np.zeros((112, 116), np.float32)
    fw2[0:56, 0:58] = fwA
    fw2[56:112, 58:116] = fwA

    # H-fft stationaries, moving rows = [Yr(h) ; Yi(h)] (112), 8 zero pad.
    fhr = np.zeros((112, 64), np.float32)
    fhr[:H, :H] = Ch.T
    fhr[H:, :H] = Sh.T
    fhi = np.zeros((112, 64), np.float32)
    fhi[:H, :H] = -Sh.T
    fhi[H:, :H] = Ch.T

    # iH stationaries, half-zeroed per quarter; psum rows = [Vr ; Vi].
    ghrA = np.zeros((H, 112), np.float32)
    ghrA[:, :H] = Chi.T
    ghrA[:, H:] = Shi.T
    ghiA = np.zeros((H, 112), np.float32)
    ghiA[:, :H] = -Shi.T
    ghiA[:, H:] = Chi.T
    ghra = np.zeros((112, 112), np.float32)
    ghra[0:56] = ghrA
    ghrb = np.zeros((112, 112), np.float32)
    ghrb[56:112] = ghrA
    ghia = np.zeros((112, 112), np.float32)
    ghia[0:56] = ghiA
    ghib = np.zeros((112, 112), np.float32)
    ghib[56:112] = ghiA

    # iW stationary, block-diag.
    gwA = np.zeros((58, W), np.float32)
    gwA[:WF] = A.T
    gwA[WF:] = Bm.T
    gw2 = np.zeros((116, 112), np.float32)
    gw2[0:58, 0:56] = gwA
    gw2[58:116, 56:112] = gwA

    # dual identity for S5 transposes.
    idd2 = np.zeros((128, 112), np.float32)
    for i in range(56):
        idd2[i, i] = 1.0
        idd2[64 + i, 56 + i] = 1.0

    dft = np.zeros((128, DFT_W), np.float32)
    for name, mat in (("fw2", fw2), ("fhr", fhr), ("fhi", fhi),
                      ("ghra", ghra), ("ghrb", ghrb), ("ghia", ghia),
                      ("ghib", ghib), ("gw2", gw2), ("idd2", idd2)):
        off, wd = DFT_COLS[name]
        dft[0:mat.shape[0], off:off + wd] = mat

    # Mixing weights, complex-interleaved on both sides, pre-transposed to
    # contraction-major (128, NB, 3, 3, 128) so the load is contiguous.
    def pack_mix(wl):
        wr, wi = wl[0], wl[1]  # (NB, 192, 192)
        m = np.zeros((NB, 3, 3, 128, 128), np.float32)
        for blk in range(NB):
            for kc in range(3):
                ds = slice(64 * kc, 64 * kc + 64)
                for mc in range(3):
                    ks = slice(64 * mc, 64 * mc + 64)
                    blkr = wr[blk][ds, ks]
                    blki = wi[blk][ds, ks]
                    t = m[blk, kc, mc]
                    t[0::2, 0::2] = blkr
                    t[1::2, 0::2] = -blki
                    t[0::2, 1::2] = blki
                    t[1::2, 1::2] = blkr
        return m.transpose((3, 0, 1, 2, 4))  # (128, NB, 3, 3, 128)

    m12 = np.stack([pack_mix(w1), pack_mix(w2)], axis=1)  # (128, 2, NB,3,3,128)

    def pack_bias(bl, scale=1.0, off=0.0):
        out = np.zeros((NB * 3, 128), np.float32)
        for blk in range(NB):
            for mc in range(3):
                ks = slice(64 * mc, 64 * mc + 64)
                out[blk * 3 + mc, 0::2] = scale * bl[0][blk][ks] + off
                out[blk * 3 + mc, 1::2] = scale * bl[1][blk][ks] + off
        return out.T  # (128, 12)

    bias = np.stack([
        pack_bias(b1),               # 0: b1p (relu bias)
        pack_bias(b2, 1.0, -LAM),    # 1: e1 = relu(v + b2 - lam)
        pack_bias(b2, -1.0, -LAM),   # 2: ACT form e2
        pack_bias(b2, 1.0, LAM),     # 3: DVE form e2
    ], axis=1)  # (128, 4, 12)

    cb = lambda a: np.ascontiguousarray(a.astype(BF))
    cf = lambda a: np.ascontiguousarray(a.astype(np.float32))
    return {"dft": cb(dft), "m12": cb(m12), "bias": cf(bias)}


def build_nc(n_b=BPC, dma_queues=None):
    if dma_queues is None:
        import os as _os
        dma_queues = tuple(
            _os.environ.get("KV3_DMA_QUEUES", "sp,pool").split(","))
    nc = bacc.Bacc(None, target_bir_lowering=False, debug=False)

    x_ext = nc.declare_dram_parameter("x", [n_b, H, W, C], F32, isOutput=False)
    out_ext = nc.declare_dram_parameter("out", [n_b, H, W, C], F32, isOutput=True)
    dft_e = nc.declare_dram_parameter("dft", [128, DFT_W], BF16, isOutput=False)
    m12_e = nc.declare_dram_parameter("m12", [128, 2, NB, 3, 3, 128], BF16,
                                      isOutput=False)
    bias_e = nc.declare_dram_parameter("bias", [128, 4, NB * 3], F32,
                                       isOutput=False)

    with tile.TileContext(nc) as tc, ExitStack() as ctx:
        consts = ctx.enter_context(tc.tile_pool(name="consts", bufs=1))
        io = ctx.enter_context(tc.tile_pool(name="io", bufs=1))
        mid = ctx.enter_context(tc.tile_pool(name="mid", bufs=1))
        mix = ctx.enter_context(tc.tile_pool(name="mix", bufs=1))
        ps = ctx.enter_context(tc.tile_pool(name="ps", bufs=1, space="PSUM"))
        dram = ctx.enter_context(tc.tile_pool(name="dram", bufs=2, space="DRAM"))

        # ---- load constants (3 DMAs, spread over queues) ----
        dft_t = consts.tile([128, DFT_W], BF16, tag="c1")
        nc.sync.dma_start(out=dft_t, in_=dft_e[:, :])
        m12_t = consts.tile([128, 2, NB, 3, 3, 128], BF16, tag="c2")
        nc.scalar.dma_start(out=m12_t, in_=m12_e[:, :, :, :, :, :])
        bias_t = consts.tile([128, 4, NB * 3], F32, tag="c3")
        nc.gpsimd.dma_start(out=bias_t, in_=bias_e[:, :, :])
        ident = consts.tile([128, 128], BF16, tag="c4")
        make_identity(nc, ident[:, :])

        def dftm(name):
            off, wd = DFT_COLS[name]
            rows = {"fw2": 112, "fhr": 112, "fhi": 112, "ghra": 112,
                    "ghrb": 112, "ghia": 112, "ghib": 112, "gw2": 116,
                    "idd2": 128}[name]
            return dft_t[0:rows, off:off + wd]

        fw2_t = dftm("fw2")
        fhr_t = dftm("fhr")
        fhi_t = dftm("fhi")
        ghra_t = dftm("ghra")
        ghrb_t = dftm("ghrb")
        ghia_t = dftm("ghia")
        ghib_t = dftm("ghib")
        gw2_t = dftm("gw2")
        idd2_t = dftm("idd2")

        # greedy engine load balancer (est. ns per engine queue)
        load = {"dve": 0.0, "act": 0.0, "pool": 0.0, "sp": 0.0}

        def pick(n, dve_fixed=170.0, act_fixed=218.0):
            cd = load["dve"] + n * 1.04 + dve_fixed
            ca = load["act"] + n * 0.833 + act_fixed
            if cd <= ca:
                load["dve"] = cd
                return "dve"
            load["act"] = ca
            return "act"

        def evict(dst, src):
            n = src.free_size()
            if pick(n) == "dve":
                nc.vector.tensor_copy(dst, src)
            else:
                nc.scalar.activation(dst, src, AF.Copy)

        DMA_QUEUES = tuple(dma_queues)

        def dma(out, in_, est):
            """Issue a DMA on the least-loaded capable queue."""
            best, cost = None, None
            for q in DMA_QUEUES:
                c = load[q] + est
                if cost is None or c < cost:
                    best, cost = q, c
            load[best] = cost
            eng = {"sp": nc.sync, "pool": nc.gpsimd, "act": nc.scalar}[best]
            eng.dma_start(out=out, in_=in_)

        for b in range(n_b):
            for g in range(2):
                cqa, cqb = 2 * g, 2 * g + 1
                cs_a = slice(cqa * BS, cqa * BS + BS)
                cs_b = slice(cqb * BS, cqb * BS + BS)

                # ---- S1+S2+rot1: load x h-chunks, Wfft, bounce to DRAM,
                # transposing read-back
                s1 = dram.tile([116, H, BS], BF16, tag="s1")
                yh = [mid.tile([112, WF, BS], BF16, tag=f"h{Hh}",
                               name=f"yh{b}{g}{Hh}") for Hh in range(2)]
                for ch in range(NCH):
                    hs = slice(HCH * ch, HCH * ch + HCH)
                    xw = io.tile([112, HCH, BS], BF16, tag="xw", bufs=2,
                                 name=f"xw{b}{g}{ch}")
                    # cast f32->bf16: must be gpsimd
                    nc.gpsimd.dma_start(
                        out=xw[0:56, :, :],
                        in_=x_ext[b, hs, :, cs_a].transpose((1, 0, 2)))
                    nc.gpsimd.dma_start(
                        out=xw[56:112, :, :],
                        in_=x_ext[b, hs, :, cs_b].transpose((1, 0, 2)))
                    load["pool"] += 2 * 4145.0
                    xw_f = xw[:, :, :].rearrange("w h c -> w (h c)")
                    ywc = io.tile([116, HCH, BS], BF16, tag="yw", bufs=2,
                                  name=f"yw{b}{g}{ch}")
                    ywc_f = ywc[:, :, :].rearrange("p h c -> p (h c)")
                    for sp in range(3):
                        pw = ps.tile([128, 2, 512], F32, tag="A", bufs=4,
                                     name=f"pw{b}{g}{ch}{sp}")
                        for half in range(2):
                            s = 2 * sp + half
                            sl = slice(448 * s, 448 * (s + 1))
                            nc.tensor.matmul(pw[0:116, half, 0:448], fw2_t,
                                             xw_f[:, sl], start=True,
                                             stop=True)
                        evict(ywc_f[:, 896 * sp:896 * (sp + 1)],
                              pw[0:116, :, 0:448])
                    dma(s1[:, hs, :], ywc, 2073.0)
                for Hh, ro in ((0, 0), (1, 58)):
                    for r in range(2):
                        dma(yh[Hh][56 * r:56 * r + 56, :, :],
                            s1[ro + 29 * r:ro + 29 * r + 29, :, :]
                            .transpose((1, 0, 2)), 8587.0)
                yh_f = [t[:, :, :].rearrange("p w c -> p (w c)") for t in yh]

                # ---- S4: Hfft -> XF [128=(a,0,b,0) | w', c, r]
                xf = mid.tile([128, WF, BS, 2], BF16, tag="big",
                              name=f"xf{b}{g}")
                xf_ri = xf[:, :, :, :].rearrange("q w c r -> q r (w c)")
                for j in range(11):
                    lo = 512 * j
                    hi = min(512 * (j + 1), NWC)
                    n = hi - lo
                    sl = slice(lo, hi)
                    pq = ps.tile([128, 2, 512], F32, tag="A", bufs=4,
                                 name=f"pq{b}{g}{j}")
                    nc.tensor.matmul(pq[0:64, 0, :n], fhr_t, yh_f[0][:, sl],
                                     start=True, stop=True)
                    nc.tensor.matmul(pq[64:128, 0, :n], fhr_t, yh_f[1][:, sl],
                                     start=True, stop=True)
                    nc.tensor.matmul(pq[0:64, 1, :n], fhi_t, yh_f[0][:, sl],
                                     start=True, stop=True)
                    nc.tensor.matmul(pq[64:128, 1, :n], fhi_t, yh_f[1][:, sl],
                                     start=True, stop=True)
                    evict(xf_ri[:, :, sl], pq[:, :, :n])

                # ---- S5: corner turn -> XM[kc] [128=(2c+r) | (H, w', h)]
                xm = []
                for kc in range(3):
                    t = mix.tile([128, 2, WF, H], BF16, tag=f"xm{kc}", bufs=2,
                                 name=f"xm{b}{g}{kc}")
                    xm.append(t)
                    src = xf[0:120, :, :, :].rearrange("q w c r -> q w (c r)")
                    dst = t[:, :, :, :]
                    wp = 0
                    for grp in (8, 8, 8, 5):
                        if wp >= WF:
                            break
                        npx = min(grp, WF - wp)
                        pt = ps.tile([128, 8, 128], F32, tag="A", bufs=4,
                                     name=f"pt{b}{g}{kc}{wp}")
                        for i in range(npx):
                            nc.tensor.matmul(
                                pt[:, i, 0:112], src[:, wp + i,
                                                     128 * kc:128 * kc + 128],
                                idd2_t[0:120, :], start=True, stop=True)
                        evict(dst[:, :, wp:wp + npx, :].transpose((0, 2, 1, 3)),
                              pt[:, 0:npx, 0:112])
                        wp += npx
                xm_f = [t[:, :, :, :].rearrange("p H w h -> p (H w h)")
                        for t in xm]

                # ---- M1/M2 per half
                hm = [mix.tile([128, 2, WF, H], BF16, tag=f"hm{kc}", bufs=1,
                               name=f"hm{b}{g}{kc}") for kc in range(3)]
                hm_f = [t[:, :, :, :].rearrange("p H w h -> p (H w h)")
                        for t in hm]
                om = [mix.tile([128, WF, 2, H], BF16, tag=f"xm{kc}", bufs=2,
                               name=f"om{b}{g}{kc}") for kc in range(3)]
                JT = (448, 448, 448, 280)
                JW = (8, 8, 8, 5)
                for Hh in range(2):
                    cq = 2 * g + Hh
                    base = SPA * Hh
                    for mc in range(3):
                        bidx = cq * 3 + mc
                        for jp in range(2):
                            n0 = JT[2 * jp]
                            n1 = JT[2 * jp + 1]
                            lo = base + 896 * jp
                            pm = ps.tile([128, 2, 512], F32, tag="A", bufs=4,
                                         name=f"pm{b}{g}{Hh}{mc}{jp}")
                            for half, nn in ((0, n0), (1, n1)):
                                sl = slice(lo + 448 * half,
                                           lo + 448 * half + nn)
                                for kc in range(3):
                                    nc.tensor.matmul(
                                        pm[:, half, :nn],
                                        m12_t[:, 0, cq, kc, mc, :],
                                        xm_f[kc][:, sl],
                                        start=(kc == 0), stop=(kc == 2))
                            n = n0 + n1
                            if n1 == n0:
                                esrc = pm[:, :, 0:n0]
                                edst = hm_f[mc][:, lo:lo + n]
                                if pick(n) == "dve":
                                    nc.vector.tensor_scalar(
                                        edst, esrc,
                                        bias_t[:, 0, bidx:bidx + 1], 0.0,
                                        ALU.add, ALU.max)
                                else:
                                    nc.scalar.activation(
                                        edst, esrc, AF.Relu,
                                        bias=bias_t[:, 0, bidx:bidx + 1],
                                        scale=1.0)
                            else:
                                for half, nn in ((0, n0), (1, n1)):
                                    esrc = pm[:, half, :nn]
                                    edst = hm_f[mc][:, lo + 448 * half:
                                                    lo + 448 * half + nn]
                                    if pick(nn) == "dve":
                                        nc.vector.tensor_scalar(
                                            edst, esrc,
                                            bias_t[:, 0, bidx:bidx + 1], 0.0,
                                            ALU.add, ALU.max)
                                    else:
                                        nc.scalar.activation(
                                            edst, esrc, AF.Relu,
                                            bias=bias_t[:, 0, bidx:bidx + 1],
                                            scale=1.0)
                for Hh in range(2):
                    cq = 2 * g + Hh
                    base = SPA * Hh
                    for mc in range(3):
                        bidx = cq * 3 + mc
                        for jp in range(2):
                            n0 = JT[2 * jp]
                            n1 = JT[2 * jp + 1]
                            lo = base + 896 * jp
                            nw = (n0 + n1) // H  # 16 or 13 w' columns
                            pm = ps.tile([128, 2, 512], F32, tag="A", bufs=4,
                                         name=f"qm{b}{g}{Hh}{mc}{jp}")
                            for half, nn in ((0, n0), (1, n1)):
                                sl = slice(lo + 448 * half,
                                           lo + 448 * half + nn)
                                for kc in range(3):
                                    nc.tensor.matmul(
                                        pm[:, half, :nn],
                                        m12_t[:, 1, cq, kc, mc, :],
                                        hm_f[kc][:, sl],
                                        start=(kc == 0), stop=(kc == 2))
                            # ragged pair handled as [2, 448]+[2, 280] APs is
                            # not rectangular; use per-half APs packed into
                            # one op via the om dst (w'-aligned):
                            e2 = mix.tile([128, 2, 448], BF16, tag="e2",
                                          bufs=2, name=f"e2{b}{g}{Hh}{mc}{jp}")
                            omd = om[mc][:, 16 * jp:16 * jp + nw, Hh, :]
                            if n0 == n1:
                                esrc = pm[:, :, 0:448]
                                e2v = e2[:, :, 0:448]
                                n = n0 + n1
                                # e1 = relu(v+b2-lam) -> om
                                if pick(n) == "dve":
                                    nc.vector.tensor_scalar(
                                        omd, esrc,
                                        bias_t[:, 1, bidx:bidx + 1], 0.0,
                                        ALU.add, ALU.max)
                                else:
                                    nc.scalar.activation(
                                        omd, esrc, AF.Relu,
                                        bias=bias_t[:, 1, bidx:bidx + 1],
                                        scale=1.0)
                                if pick(n) == "dve":
                                    nc.vector.tensor_scalar(
                                        e2v, esrc,
                                        bias_t[:, 3, bidx:bidx + 1], 0.0,
                                        ALU.add, ALU.min)
                                    cop = ALU.add
                                else:
                                    nc.scalar.activation(
                                        e2v, esrc, AF.Relu,
                                        bias=bias_t[:, 2, bidx:bidx + 1],
                                        scale=-1.0)
                                    cop = ALU.subtract
                                load["dve"] += n * 1.04 + 170
                                nc.vector.tensor_tensor(omd, omd, e2v, cop)
                            else:
                                for half, nn in ((0, n0), (1, n1)):
                                    esrc = pm[:, half, :nn]
                                    e2v = e2[:, half, :nn]
                                    omh = om[mc][:, 16 * jp + 8 * half:
                                                 16 * jp + 8 * half + nn // H,
                                                 Hh, :]
                                    if pick(nn) == "dve":
                                        nc.vector.tensor_scalar(
                                            omh, esrc,
                                            bias_t[:, 1, bidx:bidx + 1], 0.0,
                                            ALU.add, ALU.max)
                                    else:
                                        nc.scalar.activation(
                                            omh, esrc, AF.Relu,
                                            bias=bias_t[:, 1, bidx:bidx + 1],
                                            scale=1.0)
                                    if pick(nn) == "dve":
                                        nc.vector.tensor_scalar(
                                            e2v, esrc,
                                            bias_t[:, 3, bidx:bidx + 1], 0.0,
                                            ALU.add, ALU.min)
                                        cop = ALU.add
                                    else:
                                        nc.scalar.activation(
                                            e2v, esrc, AF.Relu,
                                            bias=bias_t[:, 2, bidx:bidx + 1],
                                            scale=-1.0)
                                        cop = ALU.subtract
                                    load["dve"] += nn * 1.04 + 170
                                    nc.vector.tensor_tensor(omh, omh, e2v,
                                                            cop)

                # ---- S8: corner turn back -> OC [112=(a h'|b h') | w', c, r]
                oc = mid.tile([112, WF, BS, 2], BF16, tag="big",
                              name=f"oc{b}{g}")
                for mc in range(3):
                    wp = 0
                    for grp in (8, 8, 8, 5):
                        if wp >= WF:
                            break
                        npx = min(grp, WF - wp)
                        pt8 = ps.tile([112, 8, 128], BF16, tag="A", bufs=4,
                                      name=f"p8{b}{g}{mc}{wp}")
                        for i in range(npx):
                            nc.tensor.matmul(
                                pt8[:, i, :],
                                om[mc][:, wp + i, :, :]
                                .rearrange("p H h -> p (H h)"),
                                ident[:, :], is_transpose=True)
                        evict(oc[:, wp:wp + npx, 64 * mc:64 * mc + 64, :],
                              pt8[:, 0:npx, :].rearrange("p w q -> p (w q)"))
                        wp += npx
                oc_r = oc[:, :, :, 0].rearrange("q w c -> q (w c)")
                oc_i = oc[:, :, :, 1].rearrange("q w c -> q (w c)")

                # ---- S9: iH -> VH_a/b ((r,h) | w', c)
                vh = []
                for Hh, gr, gi in ((0, ghra_t, ghia_t), (1, ghrb_t, ghib_t)):
                    t = mid.tile([112, WF, BS], BF16, tag=f"h{Hh}",
                                 name=f"vh{b}{g}{Hh}")
                    t_f = t[:, :, :].rearrange("p w c -> p (w c)")
                    for jp in range(6):
                        lo = 1024 * jp
                        hi = min(1024 * (jp + 1), NWC)
                        pv = ps.tile([128, 2, 512], F32, tag="A", bufs=4,
                                     name=f"pv{b}{g}{Hh}{jp}")
                        for half in range(2):
                            l2 = lo + 512 * half
                            h2 = min(l2 + 512, NWC)
                            if l2 >= h2:
                                continue
                            sl = slice(l2, h2)
                            nc.tensor.matmul(pv[0:112, half, :h2 - l2], gr,
                                             oc_r[:, sl],
                                             start=True, stop=False)
                            nc.tensor.matmul(pv[0:112, half, :h2 - l2], gi,
                                             oc_i[:, sl],
                                             start=False, stop=True)
                        if hi - lo == 1024:
                            evict(t_f[:, lo:hi], pv[0:112, :, :])
                        else:
                            evict(t_f[:, lo:hi],
                                  pv[0:112, :, :].rearrange("p a b -> p (a b)")
                                  [:, 0:hi - lo])
                    vh.append(t)

                # ---- rot4: VH -> DRAM s4y (Hh, r, w', h, c); store pays the
                # transpose so the per-chunk read back is nearly contiguous.
                s4y = dram.tile([2, 2, WF, H, BS], BF16, tag="s4")
                for Hh in range(2):
                    for r in range(2):
                        dma(s4y[Hh, r, :, :, :].transpose((1, 0, 2)),
                            vh[Hh][56 * r:56 * r + 56, :, :], 8587.0)
                for ch in range(NCH):
                    hs = slice(HCH * ch, HCH * ch + HCH)
                    vwc = io.tile([116, HCH, BS], BF16, tag="vw", bufs=2,
                                  name=f"vw{b}{g}{ch}")
                    dma(vwc[:, :, :], s4y[:, :, :, hs, :], 2073.0)
                    vwc_f = vwc[:, :, :].rearrange("p h c -> p (h c)")
                    outc = io.tile([112, HCH, BS], F32, tag="out", bufs=2,
                                   name=f"out{b}{g}{ch}")
                    outc_f = outc[:, :, :].rearrange("w h c -> w (h c)")
                    for sp in range(3):
                        po = ps.tile([128, 2, 512], F32, tag="A", bufs=4,
                                     name=f"po{b}{g}{ch}{sp}")
                        for half in range(2):
                            s = 2 * sp + half
                            sl = slice(448 * s, 448 * (s + 1))
                            nc.tensor.matmul(po[0:112, half, 0:448], gw2_t,
                                             vwc_f[:, sl], start=True,
                                             stop=True)
                        evict(outc_f[:, 896 * sp:896 * (sp + 1)],
                              po[0:112, :, 0:448])
                    dma(out_ext[b, hs, :, cs_a].transpose((1, 0, 2)),
                        outc[0:56, :, :], 4145.0)
                    dma(out_ext[b, hs, :, cs_b].transpose((1, 0, 2)),
                        outc[56:112, :, :], 4145.0)

    nc.compile()
    return nc


_NC_CACHE = {}


def _get_nc(n_b=BPC, dma_queues=None):
    key = (n_b, dma_queues)
    if key not in _NC_CACHE:
        _NC_CACHE[key] = build_nc(n_b, dma_queues)
    return _NC_CACHE[key]


def kernel(x, w1, b1, w2, b2):
    x = np.ascontiguousarray(np.asarray(x, dtype=np.float32))
    B, N, Cc = x.shape
    consts = make_consts(np.asarray(w1), np.asarray(b1),
                         np.asarray(w2), np.asarray(b2))
    nc = _get_nc(BPC)
    in_maps = []
    for core in range(NCORES):
        shard = np.ascontiguousarray(
            x[core * BPC:(core + 1) * BPC].reshape(BPC, H, W, Cc))
        m = {"x": shard}
        m.update(consts)
        in_maps.append(m)
    res = run_bass_kernel_spmd(nc, in_maps, core_ids=list(range(NCORES)))
    out = np.concatenate(
        [res.results[i]["out"].reshape(BPC, N, Cc) for i in range(NCORES)],
        axis=0)
    return out.astype(np.float32)
